# revision 1
# baseline (speedup 1.0000x reference)
"""Trainium2 Bass kernel for a 2-layer GATv2 + top-k pooling + classifier.

Distribution (8 NeuronCores): partition nodes (and their incoming edges)
across cores; layer-1 source features are computed replicated (x is
replicated), layer-2 source features are exchanged with one AllGather.
GAT weights / classifier are replicated.

Self-contained: only needs concourse (Bass), numpy, ml_dtypes.
"""

import numpy as np
import ml_dtypes

import concourse.bass as bass
import concourse.bacc as bacc
import concourse.mybir as mybir
import concourse.tile as tile
from concourse.bass import AP
from concourse.bass_utils import run_bass_kernel_spmd
from concourse.masks import make_identity

P = 128
NCORES = 8
SPLIT = 32768          # int16 gather index limit per table
NEG_SLOPE = 0.2
TOPK = 10

bf16 = mybir.dt.bfloat16
f32 = mybir.dt.float32
i16 = mybir.dt.int16

_bf = ml_dtypes.bfloat16


def _wrap_idx(idx):
    """Pack int16 indices into dma_gather's [128, n//16] SBUF layout."""
    n = idx.shape[0]
    assert n % 16 == 0
    t = idx.astype(np.int16).reshape(n // 16, 16).T
    return np.tile(t, (8, 1))


def _pad(a, n, val=0):
    out = np.full(n, val, dtype=np.int64)
    out[: len(a)] = a
    return out


def _ceil(a, b):
    return -(-a // b)


def _bcast_mid(ap, ct):
    """[P, d] AP -> [P, ct, d] AP with broadcast (step-0) middle dim."""
    return AP(ap.tensor, ap.offset, [ap.ap[0], [0, ct], ap.ap[1]])


def _prep_graph(src, dst, n_nodes):
    """Partition edges by dst core, sort by dst, tile dsts by 128, split
    sources at SPLIT. Returns per-core packed index/selection inputs plus
    the (shared) per-tile chunk counts."""
    npc = n_nodes // NCORES                # nodes per core
    ntile = _ceil(npc, P)                  # dst tiles per core
    core_of = dst // npc

    pc = []
    for c in range(NCORES):
        m = core_of == c
        es = src[m].astype(np.int64)
        ed = dst[m].astype(np.int64) - c * npc
        o = np.argsort(ed, kind="stable")
        es, ed = es[o], ed[o]
        bounds = np.searchsorted(ed, np.arange(0, ntile * P + 1, P))
        tiles = []
        for t in range(ntile):
            sl = slice(bounds[t], bounds[t + 1])
            ts_, td_ = es[sl], ed[sl]
            am = ts_ < SPLIT
            tiles.append(dict(
                a_src=ts_[am], a_fd=td_[am], a_col=td_[am] - t * P,
                b_src=ts_[~am] - SPLIT, b_fd=td_[~am], b_col=td_[~am] - t * P,
            ))
        pc.append(tiles)

    cA = [max(_ceil(len(pc[c][t]["a_src"]), P) for c in range(NCORES))
          for t in range(ntile)]
    cB = [max(_ceil(len(pc[c][t]["b_src"]), P) for c in range(NCORES))
          for t in range(ntile)]
    C = [cA[t] + cB[t] for t in range(ntile)]

    per_core = []
    for c in range(NCORES):
        iA, iB, iF, Ss = [], [], [], []
        for t in range(ntile):
            d = pc[c][t]
            nA, nB = len(d["a_src"]), len(d["b_src"])
            if cA[t]:
                iA.append(_wrap_idx(_pad(d["a_src"], cA[t] * P)))
            if cB[t]:
                iB.append(_wrap_idx(_pad(d["b_src"], cB[t] * P)))
            S3 = np.zeros((C[t] * P, P), dtype=np.float32)
            S3[np.arange(nA), d["a_col"]] = 1.0
            S3[cA[t] * P + np.arange(nB), d["b_col"]] = 1.0
            Ss.append(
                S3.reshape(C[t], P, P).transpose(1, 0, 2).reshape(P, C[t] * P))
            iF.append(
                S3.reshape(C[t], P, P).transpose(2, 0, 1).reshape(P, C[t] * P))
        per_core.append(dict(
            iA=np.concatenate(iA, axis=1) if iA else np.zeros((P, 8), np.int16),
            iB=np.concatenate(iB, axis=1) if iB else np.zeros((P, 8), np.int16),
            ST=np.concatenate(iF, axis=1).astype(_bf),
            S=np.concatenate(Ss, axis=1).astype(_bf),
        ))
    return per_core, cA, cB, C, ntile, npc


def _aug(w, b):
    return np.vstack([np.asarray(w), np.asarray(b)[None, :]])


def build_nc(meta, stop_after="full"):
    n_nodes = meta["n_nodes"]
    npc = meta["npc"]
    ntile = meta["ntile"]
    cA, cB, C = meta["cA"], meta["cB"], meta["C"]
    Cmax = max(C)
    sumA, sumB, sumC = sum(cA), sum(cB), sum(C)
    emb = meta["emb"]
    dd = meta["d"]
    H = meta["H"]
    F = dd // H
    ncls = meta["ncls"]
    npc_pad = ntile * P
    nfull = meta["nfull_pad"]
    rowsA = min(nfull, SPLIT)
    rowsB_pad = max(_ceil(nfull - rowsA, P) * P, P)
    ngrp = npc // TOPK
    use_B = n_nodes > SPLIT
    nfull_ag_pad = _ceil(n_nodes, P) * P

    nc = bacc.Bacc(num_swdge_queues=4)

    xTa = nc.declare_dram_parameter("xTa", [emb + 1, nfull], bf16, isOutput=False)
    xTl = nc.declare_dram_parameter("xTl", [emb + 1, npc_pad], bf16, isOutput=False)
    w1s = nc.declare_dram_parameter("w1s", [emb + 1, dd], bf16, isOutput=False)
    w1d = nc.declare_dram_parameter("w1d", [emb + 1, dd], bf16, isOutput=False)
    w2s = nc.declare_dram_parameter("w2s", [dd + 1, dd], bf16, isOutput=False)
    w2d = nc.declare_dram_parameter("w2d", [dd + 1, dd], bf16, isOutput=False)
    a1r = nc.declare_dram_parameter("a1r", [P, dd], bf16, isOutput=False)
    a2r = nc.declare_dram_parameter("a2r", [P, dd], bf16, isOutput=False)
    wca = nc.declare_dram_parameter("wca", [dd + 1, ncls], f32, isOutput=False)
    pwp = nc.declare_dram_parameter("pwp", [P, 5], f32, isOutput=False)
    iA_in = nc.declare_dram_parameter("iA", [P, max(sumA, 1) * 8], i16, isOutput=False)
    iB_in = nc.declare_dram_parameter("iB", [P, max(sumB, 1) * 8], i16, isOutput=False)
    ST_in = nc.declare_dram_parameter("ST", [P, sumC * P], bf16, isOutput=False)
    S_in = nc.declare_dram_parameter("S", [P, sumC * P], bf16, isOutput=False)
    out = nc.declare_dram_parameter("out", [ngrp, ncls], f32, isOutput=True)

    fs1a = nc.dram_tensor("fs1a", [rowsA, dd], bf16)
    fs1b = nc.dram_tensor("fs1b", [rowsB_pad, dd], bf16)
    fd1 = nc.dram_tensor("fd1", [npc_pad, dd], bf16)
    fd2 = nc.dram_tensor("fd2", [npc_pad, dd], bf16)
    fs2l = nc.dram_tensor("fs2l", [npc, dd], bf16)
    fs2f = nc.dram_tensor("fs2f", [nfull_ag_pad, dd], bf16, addr_space="Shared")

    AF = mybir.ActivationFunctionType
    ALU = mybir.AluOpType
    BLK = 1024

    with tile.TileContext(nc) as tc:
        with (
            tc.tile_pool(name="const", bufs=1) as cpool,
            tc.tile_pool(name="wpool", bufs=1) as wpool,
            tc.tile_pool(name="xload", bufs=2) as xpool,
            tc.tile_pool(name="mmout", bufs=3) as mpool,
            tc.tile_pool(name="edgeg", bufs=3) as epool,
            tc.tile_pool(name="vp", bufs=2) as vpool,
            tc.tile_pool(name="sp2", bufs=2) as s2pool,
            tc.tile_pool(name="zp", bufs=1) as zpool,
            tc.tile_pool(name="small", bufs=3) as spool,
            tc.tile_pool(name="hbuf", bufs=1) as hpool,
            tc.tile_pool(name="psA", bufs=2, space="PSUM") as psA,
            tc.tile_pool(name="psT", bufs=2, space="PSUM") as psT,
            tc.tile_pool(name="psE", bufs=2, space="PSUM") as psE,
            tc.tile_pool(name="psF", bufs=2, space="PSUM") as psF,
        ):
            ones1 = cpool.tile([1, P], bf16)
            nc.vector.memset(ones1[:], 1.0)
            ones1f = cpool.tile([1, P], f32)
            nc.vector.memset(ones1f[:], 1.0)
            ident = cpool.tile([P, P], bf16)
            make_identity(nc, ident[:])
            a1t = cpool.tile([P, dd], bf16)
            nc.sync.dma_start(out=a1t[:], in_=a1r[:])
            a2t = cpool.tile([P, dd], bf16)
            nc.sync.dma_start(out=a2t[:], in_=a2r[:])
            pw = cpool.tile([P, 5], f32)
            nc.sync.dma_start(out=pw[:], in_=pwp[:])

            def load_w(src_t, kdim, nm):
                t0 = wpool.tile([P, dd], bf16, tag=nm + "0")
                t1 = wpool.tile([P, dd], bf16, tag=nm + "1")
                t2 = wpool.tile([1, dd], bf16, tag=nm + "2")
                nc.sync.dma_start(out=t0[:], in_=src_t[0:P, :])
                nc.sync.dma_start(out=t1[:], in_=src_t[P:2 * P, :])
                nc.sync.dma_start(out=t2[:], in_=src_t[kdim:kdim + 1, :])
                return t0, t1, t2

            w1s_t = load_w(w1s, emb, "w1s")
            w1d_t = load_w(w1d, emb, "w1d")
            w2s_t = load_w(w2s, dd, "w2s")
            w2d_t = load_w(w2d, dd, "w2d")

            def mm_rows(x0, x1, m0, wtile, psum):
                nc.tensor.matmul(out=psum[:], lhsT=x0[:, m0:m0 + P],
                                 rhs=wtile[0][:], start=True, stop=False)
                nc.tensor.matmul(out=psum[:], lhsT=x1[:, m0:m0 + P],
                                 rhs=wtile[1][:], start=False, stop=False)
                nc.tensor.matmul(out=psum[:], lhsT=ones1[:], rhs=wtile[2][:],
                                 start=False, stop=True)

            # ---------- phase A: fd1 (local) first, then fs1a, then fs1b ----
            for b in range(_ceil(npc_pad, BLK)):
                w = min(BLK, npc_pad - b * BLK)
                x0 = xpool.tile([P, BLK], bf16, tag="x0")
                x1 = xpool.tile([P, BLK], bf16, tag="x1")
                nc.sync.dma_start(out=x0[:, :w], in_=xTl[0:P, b * BLK:b * BLK + w])
                nc.sync.dma_start(out=x1[:, :w], in_=xTl[P:2 * P, b * BLK:b * BLK + w])
                for m in range(w // P):
                    row0 = b * BLK + m * P
                    ps = psA.tile([P, dd], f32, space="PSUM", tag="psa")
                    mm_rows(x0, x1, m * P, w1d_t, ps)
                    ot = mpool.tile([P, dd], bf16, tag="fsout")
                    nc.scalar.copy(out=ot[:], in_=ps[:])
                    nc.sync.dma_start(out=fd1[row0:row0 + P, :], in_=ot[:])

            for part in (0, 1):
                lo = 0 if part == 0 else rowsA
                hi = rowsA if part == 0 else nfull
                for b in range(lo // BLK, _ceil(hi, BLK)):
                    w = min(BLK, nfull - b * BLK)
                    x0 = xpool.tile([P, BLK], bf16, tag="x0")
                    x1 = xpool.tile([P, BLK], bf16, tag="x1")
                    nc.sync.dma_start(out=x0[:, :w],
                                      in_=xTa[0:P, b * BLK:b * BLK + w])
                    nc.sync.dma_start(out=x1[:, :w],
                                      in_=xTa[P:2 * P, b * BLK:b * BLK + w])
                    for m in range(w // P):
                        row0 = b * BLK + m * P
                        if (row0 < rowsA) != (part == 0):
                            continue
                        ps = psA.tile([P, dd], f32, space="PSUM", tag="psa")
                        mm_rows(x0, x1, m * P, w1s_t, ps)
                        ot = mpool.tile([P, dd], bf16, tag="fsout")
                        nc.scalar.copy(out=ot[:], in_=ps[:])
                        if row0 < rowsA:
                            nc.sync.dma_start(out=fs1a[row0:row0 + P, :],
                                              in_=ot[:])
                        else:
                            r = row0 - rowsA
                            nc.sync.dma_start(out=fs1b[r:r + P, :], in_=ot[:])

            # ---------- edge phase ----------
            h1 = hpool.tile([P, ntile, dd], bf16, tag="h")
            h2 = hpool.tile([P, ntile, dd], bf16, tag="h")

            def edge_phase(tabA, tabB, tabF, a_t, hdst, scale_posw, depth="all"):
                offA = offB = offC = 0
                for t in range(ntile):
                    ca, cb, ct = cA[t], cB[t], C[t]
                    St = s2pool.tile([P, Cmax * P], bf16, tag="S")
                    nc.sync.dma_start(
                        out=St[:, : ct * P],
                        in_=S_in[:, offC * P:(offC + ct) * P])
                    E = epool.tile([P, Cmax, dd], bf16, tag="E")
                    if ca:
                        ia = spool.tile([P, Cmax * 8], i16, tag="ia")
                        nc.sync.dma_start(
                            out=ia[:, : ca * 8],
                            in_=iA_in[:, offA * 8:(offA + ca) * 8])
                        ah = (ca + 1) // 2
                        nc.gpsimd.dma_gather(
                            out_ap=E[:, 0:ah, :], in_ap=tabA,
                            idxs_ap=ia[:, : ah * 8], num_idxs=ah * P,
                            num_idxs_reg=ah * P, elem_size=dd,
                            single_packet=False, queue_num=(3 * t) % 4)
                        if ca > ah:
                            nc.gpsimd.dma_gather(
                                out_ap=E[:, ah:ca, :], in_ap=tabA,
                                idxs_ap=ia[:, ah * 8: ca * 8],
                                num_idxs=(ca - ah) * P,
                                num_idxs_reg=(ca - ah) * P, elem_size=dd,
                                single_packet=False, queue_num=(3 * t + 1) % 4)
                    if cb:
                        ib = spool.tile([P, Cmax * 8], i16, tag="ib")
                        nc.sync.dma_start(
                            out=ib[:, : cb * 8],
                            in_=iB_in[:, offB * 8:(offB + cb) * 8])
                        nc.gpsimd.dma_gather(
                            out_ap=E[:, ca:ct, :], in_ap=tabB,
                            idxs_ap=ib[:, : cb * 8], num_idxs=cb * P,
                            num_idxs_reg=cb * P, elem_size=dd,
                            single_packet=False, queue_num=(3 * t + 2) % 4)
                    # fd broadcast: fd rows for this tile's 128 dsts, expanded
                    # to edge slots via the transposed one-hot (PE matmul)
                    STt = s2pool.tile([P, Cmax * P], bf16, tag="ST")
                    nc.sync.dma_start(
                        out=STt[:, : ct * P],
                        in_=ST_in[:, offC * P:(offC + ct) * P])
                    fdt = spool.tile([P, dd], bf16, tag="fdt")
                    nc.sync.dma_start(out=fdt[:], in_=tabF[t * P:(t + 1) * P, :])
                    w_ = ct * dd
                    LZ = zpool.tile([P, Cmax * dd], bf16, tag="LZ")
                    for c in range(ct):
                        psf = psF.tile([P, dd], f32, space="PSUM", tag="psf")
                        nc.tensor.matmul(
                            out=psf[:], lhsT=STt[:, c * P:(c + 1) * P],
                            rhs=fdt[:], start=True, stop=False)
                        nc.tensor.matmul(
                            out=psf[:], lhsT=ident[:], rhs=E[:, c, :],
                            start=False, stop=True)
                        nc.scalar.activation(
                            LZ[:, c * dd:(c + 1) * dd], psf[:], AF.Prelu,
                            alpha=NEG_SLOPE)

                    if depth == "g":
                        nc.vector.memset(hdst[:, t, :], 0.0)
                        offA += ca; offB += cb; offC += ct
                        continue
                    T = zpool.tile([P, Cmax * dd], bf16, tag="T")
                    nc.vector.tensor_mul(
                        out=T[:, :w_].rearrange("p (c d) -> p c d", d=dd),
                        in0=LZ[:, :w_].rearrange("p (c d) -> p c d", d=dd),
                        in1=_bcast_mid(a_t[:], ct))
                    TF = zpool.tile([P, Cmax * dd // 2], bf16, tag="TF")
                    tv = T[:, :w_].rearrange("p (ch f) -> p ch f", f=F)
                    nc.vector.tensor_add(
                        out=TF[:, : w_ // 2].rearrange("p (ch f) -> p ch f",
                                                       f=F // 2),
                        in0=tv[:, :, 0:F // 2], in1=tv[:, :, F // 2:F])
                    score = spool.tile([P, Cmax * H], f32, tag="sc")
                    nc.vector.reduce_sum(
                        out=score[:, : ct * H],
                        in_=TF[:, : w_ // 2].rearrange("p (ch f) -> p ch f",
                                                       f=F // 2),
                        axis=mybir.AxisListType.X)
                    EX = spool.tile([P, Cmax * H], f32, tag="ex")
                    nc.scalar.activation(EX[:, : ct * H], score[:, : ct * H],
                                         AF.Exp)
                    if depth == "dve":
                        nc.vector.memset(hdst[:, t, :], 0.0)
                        offA += ca; offB += cb; offC += ct
                        continue

                    V = vpool.tile([P, Cmax, dd + H], bf16, tag="V")
                    EXB = zpool.tile([P, Cmax * dd], bf16, tag="EXB")
                    exs = EX[:, : ct * H]
                    nc.scalar.copy(
                        out=EXB[:, :w_].rearrange("p (ch f) -> p ch f", f=F),
                        in_=AP(exs.tensor, exs.offset,
                               [exs.ap[0], exs.ap[1], [0, F]]))
                    nc.vector.tensor_mul(
                        out=V[:, 0:ct, 0:dd],
                        in0=E[:, 0:ct, :],
                        in1=EXB[:, :w_].rearrange("p (c d) -> p c d", d=dd))
                    nc.scalar.copy(
                        out=V[:, 0:ct, dd:dd + H],
                        in_=EX[:, : ct * H].rearrange("p (c h) -> p c h", h=H))

                    if depth == "v":
                        nc.vector.memset(hdst[:, t, :], 0.0)
                        offA += ca; offB += cb; offC += ct
                        continue
                    agg = psE.tile([P, dd + H], f32, space="PSUM", tag="agg")
                    for c in range(ct):
                        nc.tensor.matmul(
                            out=agg[:], lhsT=St[:, c * P:(c + 1) * P],
                            rhs=V[:, c, :], start=(c == 0), stop=(c == ct - 1))

                    den = spool.tile([P, H], f32, tag="den")
                    nc.vector.tensor_scalar_max(den[:], agg[:, dd:dd + H], 1e-9)
                    rec = spool.tile([P, H], f32, tag="rec")
                    nc.vector.reciprocal(rec[:], den[:])
                    if scale_posw:
                        nc.vector.tensor_scalar_mul(rec[:], rec[:],
                                                    pw[:, t % 5:t % 5 + 1])
                    for h in range(H):
                        nc.vector.tensor_scalar_mul(
                            hdst[:, t, h * F:(h + 1) * F],
                            agg[:, h * F:(h + 1) * F], rec[:, h:h + 1])
                    offA += ca
                    offB += cb
                    offC += ct

            if stop_after != "A":
                depth = {"L1g": "g", "L1dve": "dve", "L1v": "v"}.get(
                    stop_after, "all")
                edge_phase(fs1a[:], fs1b[:], fd1, a1t, h1, False, depth)

            # ---------- transpose h1 ----------
            hT0 = hpool.tile([P, npc_pad], bf16, tag="t0")
            hT1 = hpool.tile([P, npc_pad], bf16, tag="t1")
            for t in range(ntile if stop_after in ("TR", "FS2", "AG", "full") else 0):
                for half, ht in ((0, hT0), (1, hT1)):
                    pt = psT.tile([P, P], bf16, space="PSUM", tag="ptr")
                    nc.tensor.transpose(
                        out=pt[:], in_=h1[:, t, half * P:(half + 1) * P],
                        identity=ident[:])
                    nc.scalar.copy(out=ht[:, t * P:(t + 1) * P], in_=pt[:])

            # ---------- fs2 / fd2 local ----------
            for t in range(ntile if stop_after in ("FS2", "AG", "full") else 0):
                for wt, ob in ((w2s_t, "s"), (w2d_t, "d")):
                    ps = psA.tile([P, dd], f32, space="PSUM", tag="psa")
                    nc.tensor.matmul(out=ps[:], lhsT=hT0[:, t * P:(t + 1) * P],
                                     rhs=wt[0][:], start=True, stop=False)
                    nc.tensor.matmul(out=ps[:], lhsT=hT1[:, t * P:(t + 1) * P],
                                     rhs=wt[1][:], start=False, stop=False)
                    nc.tensor.matmul(out=ps[:], lhsT=ones1[:], rhs=wt[2][:],
                                     start=False, stop=True)
                    ot = mpool.tile([P, dd], bf16, tag="fsout")
                    nc.scalar.copy(out=ot[:], in_=ps[:])
                    if ob == "s":
                        hi = min((t + 1) * P, npc)
                        if hi > t * P:
                            nc.sync.dma_start(out=fs2l[t * P:hi, :],
                                              in_=ot[: hi - t * P, :])
                    else:
                        nc.sync.dma_start(out=fd2[t * P:(t + 1) * P, :], in_=ot[:])

            # ---------- AllGather fs2 ----------
            do_rest = stop_after in ("AG", "full")
            if do_rest:
                nc.gpsimd.collective_compute(
                "AllGather", ALU.bypass,
                    replica_groups=[list(range(NCORES))],
                    ins=[fs2l[:]], outs=[fs2f[0:n_nodes, :]])

            if do_rest and stop_after == "AG":
                do_rest = False
            if do_rest and use_B:
                edge_phase(fs2f[0:SPLIT, :], fs2f[SPLIT:nfull_ag_pad, :],
                           fd2, a2t, h2, True)
            elif do_rest:
                edge_phase(fs2f[0:nfull_ag_pad, :], fs2f[:], fd2, a2t, h2, True)

            # ---------- transpose h2 ----------
            gT0 = hpool.tile([P, npc_pad], bf16, tag="t0")
            gT1 = hpool.tile([P, npc_pad], bf16, tag="t1")
            for t in range(ntile if stop_after == "full" else 0):
                for half, ht in ((0, gT0), (1, gT1)):
                    pt = psT.tile([P, P], bf16, space="PSUM", tag="ptr")
                    nc.tensor.transpose(
                        out=pt[:], in_=h2[:, t, half * P:(half + 1) * P],
                        identity=ident[:])
                    nc.scalar.copy(out=ht[:, t * P:(t + 1) * P], in_=pt[:])

            # ---------- pooling + classifier ----------
            p0 = hpool.tile([P, npc // TOPK], f32, tag="p0")
            p1 = hpool.tile([P, npc // TOPK], f32, tag="p1")
            if stop_after == "full":
                nc.vector.reduce_sum(
                    out=p0[:],
                    in_=gT0[:, :npc].rearrange("p (g k) -> p g k", k=TOPK),
                    axis=mybir.AxisListType.X)
                nc.vector.reduce_sum(
                    out=p1[:],
                    in_=gT1[:, :npc].rearrange("p (g k) -> p g k", k=TOPK),
                    axis=mybir.AxisListType.X)
            else:
                nc.vector.memset(p0[:], 0.0)
                nc.vector.memset(p1[:], 0.0)

            wc0 = wpool.tile([P, ncls], f32, tag="wc0")
            wc1 = wpool.tile([P, ncls], f32, tag="wc1")
            wc2 = wpool.tile([1, ncls], f32, tag="wc2")
            nc.sync.dma_start(out=wc0[:], in_=wca[0:P, :])
            nc.sync.dma_start(out=wc1[:], in_=wca[P:2 * P, :])
            nc.sync.dma_start(out=wc2[:], in_=wca[dd:dd + 1, :])

            for g0 in range(0, ngrp, P):
                gw = min(P, ngrp - g0)
                pc_ = psA.tile([P, ncls], f32, space="PSUM", tag="psa")
                nc.tensor.matmul(out=pc_[:gw, :], lhsT=p0[:, g0:g0 + gw],
                                 rhs=wc0[:], start=True, stop=False)
                nc.tensor.matmul(out=pc_[:gw, :], lhsT=p1[:, g0:g0 + gw],
                                 rhs=wc1[:], start=False, stop=False)
                nc.tensor.matmul(out=pc_[:gw, :], lhsT=ones1f[:, :gw],
                                 rhs=wc2[:], start=False, stop=True)
                oc = mpool.tile([P, ncls], f32, tag="ocls")
                nc.scalar.copy(out=oc[:gw, :], in_=pc_[:gw, :])
                nc.sync.dma_start(out=out[g0:g0 + gw, :], in_=oc[:gw, :])

    nc.compile()
    return nc


def _build_inputs(inputs):
    x = np.asarray(inputs["x"], dtype=np.float32)
    src = np.asarray(inputs["src"]).astype(np.int64)
    dst = np.asarray(inputs["dst"]).astype(np.int64)
    n_nodes, emb = x.shape
    dd = np.asarray(inputs["w1_src"]).shape[1]
    H = np.asarray(inputs["a1"]).shape[0]
    ncls = np.asarray(inputs["wc"]).shape[1]

    per_core, cA, cB, C, ntile, npc = _prep_graph(src, dst, n_nodes)
    npc_pad = ntile * P
    nfull_pad = _ceil(n_nodes, P) * P

    meta = dict(n_nodes=n_nodes, npc=npc, ntile=ntile, cA=cA, cB=cB, C=C,
                emb=emb, d=dd, H=H, ncls=ncls, nfull_pad=nfull_pad)

    xT = np.zeros((emb + 1, nfull_pad), dtype=_bf)
    xT[:emb, :n_nodes] = x.T.astype(_bf)
    xT[emb, :] = _bf(1.0)
    w1s = _aug(inputs["w1_src"], inputs["b1_src"]).astype(_bf)
    w1d = _aug(inputs["w1_dst"], inputs["b1_dst"]).astype(_bf)
    w2s = _aug(inputs["w2_src"], inputs["b2_src"]).astype(_bf)
    w2d = _aug(inputs["w2_dst"], inputs["b2_dst"]).astype(_bf)
    a1rr = np.tile(np.asarray(inputs["a1"]).reshape(1, -1), (P, 1)).astype(_bf)
    a2rr = np.tile(np.asarray(inputs["a2"]).reshape(1, -1), (P, 1)).astype(_bf)
    wca = _aug(inputs["wc"], inputs["bc"]).astype(np.float32)
    pos_w = np.asarray(inputs["pos_w"], dtype=np.float32)
    pwp = np.zeros((P, 5), dtype=np.float32)
    for j in range(5):
        for p in range(P):
            pwp[p, j] = pos_w[(P * j + p) % TOPK]

    in_maps = []
    for c in range(NCORES):
        d = per_core[c]
        xl = np.zeros((emb + 1, npc_pad), dtype=_bf)
        xl[:emb, :npc] = x[c * npc:(c + 1) * npc].T.astype(_bf)
        xl[emb, :] = _bf(1.0)
        in_maps.append(dict(
            xTa=xT, xTl=xl, w1s=w1s, w1d=w1d, w2s=w2s, w2d=w2d,
            a1r=a1rr, a2r=a2rr, wca=wca, pwp=pwp,
            iA=np.ascontiguousarray(d["iA"]), iB=np.ascontiguousarray(d["iB"]),
            ST=np.ascontiguousarray(d["ST"]), S=np.ascontiguousarray(d["S"]),
        ))
    return meta, in_maps


def run(inputs, trace=False, stop_after="full", cores=None):
    meta, in_maps = _build_inputs(inputs)
    nc = build_nc(meta, stop_after=stop_after)
    ids = list(range(NCORES)) if cores is None else list(range(cores))
    res = run_bass_kernel_spmd(nc, [in_maps[c] for c in ids], core_ids=ids,
                               trace=trace)
    outs = [res.results[i]["out"] for i in range(len(ids))]
    return np.concatenate(outs, axis=0), res


def kernel(**inputs):
    out, _ = run(inputs, trace=False)
    return out



# revision 6
# speedup vs baseline: 1.0912x; 1.0912x over previous
"""Trainium2 Bass kernel for a 2-layer GATv2 + top-k pooling + classifier.

Distribution (8 NeuronCores): partition nodes (and their incoming edges)
across cores; layer-1 source features are computed replicated (x is
replicated), layer-2 source features are exchanged with one AllGather.
GAT weights / classifier are replicated.

Self-contained: only needs concourse (Bass), numpy, ml_dtypes.
"""

import numpy as np
import ml_dtypes

import concourse.bass as bass
import concourse.bacc as bacc
import concourse.mybir as mybir
import concourse.tile as tile
from concourse.bass import AP
from concourse.bass_utils import run_bass_kernel_spmd
from concourse.masks import make_identity

P = 128
NCORES = 8
SPLIT = 32768          # int16 gather index limit per table
NEG_SLOPE = 0.2
TOPK = 10

bf16 = mybir.dt.bfloat16
f32 = mybir.dt.float32
i16 = mybir.dt.int16

_bf = ml_dtypes.bfloat16


def _wrap_idx(idx):
    """Pack int16 indices into dma_gather's [128, n//16] SBUF layout."""
    n = idx.shape[0]
    assert n % 16 == 0
    t = idx.astype(np.int16).reshape(n // 16, 16).T
    return np.tile(t, (8, 1))


def _pad(a, n, val=0):
    out = np.full(n, val, dtype=np.int64)
    out[: len(a)] = a
    return out


def _ceil(a, b):
    return -(-a // b)


def _bcast_mid(ap, ct):
    """[P, d] AP -> [P, ct, d] AP with broadcast (step-0) middle dim."""
    return AP(ap.tensor, ap.offset, [ap.ap[0], [0, ct], ap.ap[1]])


def _prep_graph(src, dst, n_nodes):
    """Partition edges by dst core, sort by dst, tile dsts by 128, split
    sources at SPLIT. Returns per-core packed index/selection inputs plus
    the (shared) per-tile chunk counts."""
    npc = n_nodes // NCORES                # nodes per core
    ntile = _ceil(npc, P)                  # dst tiles per core
    core_of = dst // npc

    pc = []
    for c in range(NCORES):
        m = core_of == c
        es = src[m].astype(np.int64)
        ed = dst[m].astype(np.int64) - c * npc
        o = np.argsort(ed, kind="stable")
        es, ed = es[o], ed[o]
        bounds = np.searchsorted(ed, np.arange(0, ntile * P + 1, P))
        tiles = []
        for t in range(ntile):
            sl = slice(bounds[t], bounds[t + 1])
            ts_, td_ = es[sl], ed[sl]
            am = ts_ < SPLIT
            tiles.append(dict(
                a_src=ts_[am], a_fd=td_[am], a_col=td_[am] - t * P,
                b_src=ts_[~am] - SPLIT, b_fd=td_[~am], b_col=td_[~am] - t * P,
            ))
        pc.append(tiles)

    cA = [max(_ceil(len(pc[c][t]["a_src"]), P) for c in range(NCORES))
          for t in range(ntile)]
    cB = [max(_ceil(len(pc[c][t]["b_src"]), P) for c in range(NCORES))
          for t in range(ntile)]
    C = [cA[t] + cB[t] for t in range(ntile)]

    per_core = []
    for c in range(NCORES):
        iA, iB, iF, Ss = [], [], [], []
        for t in range(ntile):
            d = pc[c][t]
            nA, nB = len(d["a_src"]), len(d["b_src"])
            if cA[t]:
                iA.append(_wrap_idx(_pad(d["a_src"], cA[t] * P)))
            if cB[t]:
                iB.append(_wrap_idx(_pad(d["b_src"], cB[t] * P)))
            S3 = np.zeros((C[t] * P, P), dtype=np.float32)
            S3[np.arange(nA), d["a_col"]] = 1.0
            S3[cA[t] * P + np.arange(nB), d["b_col"]] = 1.0
            Ss.append(
                S3.reshape(C[t], P, P).transpose(1, 0, 2).reshape(P, C[t] * P))
            iF.append(
                S3.reshape(C[t], P, P).transpose(2, 0, 1).reshape(P, C[t] * P))
        per_core.append(dict(
            iA=np.concatenate(iA, axis=1) if iA else np.zeros((P, 8), np.int16),
            iB=np.concatenate(iB, axis=1) if iB else np.zeros((P, 8), np.int16),
            ST=np.concatenate(iF, axis=1).astype(_bf),
            S=np.concatenate(Ss, axis=1).astype(_bf),
        ))
    return per_core, cA, cB, C, ntile, npc


def _aug(w, b):
    return np.vstack([np.asarray(w), np.asarray(b)[None, :]])


def build_nc(meta, stop_after="full"):
    n_nodes = meta["n_nodes"]
    npc = meta["npc"]
    ntile = meta["ntile"]
    cA, cB, C = meta["cA"], meta["cB"], meta["C"]
    Cmax = max(C)
    sumA, sumB, sumC = sum(cA), sum(cB), sum(C)
    emb = meta["emb"]
    dd = meta["d"]
    H = meta["H"]
    F = dd // H
    ncls = meta["ncls"]
    npc_pad = ntile * P
    nfull = meta["nfull_pad"]
    rowsA = min(nfull, SPLIT)
    rowsB_pad = max(_ceil(nfull - rowsA, P) * P, P)
    ngrp = npc // TOPK
    use_B = n_nodes > SPLIT
    nfull_ag_pad = _ceil(n_nodes, P) * P

    nc = bacc.Bacc(num_swdge_queues=4)

    xTa = nc.declare_dram_parameter("xTa", [emb + 1, nfull], bf16, isOutput=False)
    xTl = nc.declare_dram_parameter("xTl", [emb + 1, npc_pad], bf16, isOutput=False)
    w1s = nc.declare_dram_parameter("w1s", [emb + 1, dd], bf16, isOutput=False)
    w1d = nc.declare_dram_parameter("w1d", [emb + 1, dd], bf16, isOutput=False)
    w2s = nc.declare_dram_parameter("w2s", [dd + 1, dd], bf16, isOutput=False)
    w2d = nc.declare_dram_parameter("w2d", [dd + 1, dd], bf16, isOutput=False)
    a1r = nc.declare_dram_parameter("a1r", [P, dd], bf16, isOutput=False)
    a2r = nc.declare_dram_parameter("a2r", [P, dd], bf16, isOutput=False)
    wca = nc.declare_dram_parameter("wca", [dd + 1, ncls], f32, isOutput=False)
    pwp = nc.declare_dram_parameter("pwp", [P, 5], f32, isOutput=False)
    iA_in = nc.declare_dram_parameter("iA", [P, max(sumA, 1) * 8], i16, isOutput=False)
    iB_in = nc.declare_dram_parameter("iB", [P, max(sumB, 1) * 8], i16, isOutput=False)
    ST_in = nc.declare_dram_parameter("ST", [P, sumC * P], bf16, isOutput=False)
    S_in = nc.declare_dram_parameter("S", [P, sumC * P], bf16, isOutput=False)
    out = nc.declare_dram_parameter("out", [ngrp, ncls], f32, isOutput=True)

    fs1a = nc.dram_tensor("fs1a", [rowsA, dd], bf16)
    fs1b = nc.dram_tensor("fs1b", [rowsB_pad, dd], bf16)
    fd1 = nc.dram_tensor("fd1", [npc_pad, dd], bf16)
    fd2 = nc.dram_tensor("fd2", [npc_pad, dd], bf16)
    fs2l = nc.dram_tensor("fs2l", [npc, dd], bf16)
    fs2f = nc.dram_tensor("fs2f", [nfull_ag_pad, dd], bf16, addr_space="Shared")

    AF = mybir.ActivationFunctionType
    ALU = mybir.AluOpType
    BLK = 1024

    with tile.TileContext(nc) as tc:
        with (
            tc.tile_pool(name="const", bufs=1) as cpool,
            tc.tile_pool(name="wpool", bufs=1) as wpool,
            tc.tile_pool(name="xload", bufs=2) as xpool,
            tc.tile_pool(name="mmout", bufs=3) as mpool,
            tc.tile_pool(name="edgeg", bufs=3) as epool,
            tc.tile_pool(name="vp", bufs=2) as vpool,
            tc.tile_pool(name="sp2", bufs=2) as s2pool,
            tc.tile_pool(name="zp", bufs=1) as zpool,
            tc.tile_pool(name="small", bufs=3) as spool,
            tc.tile_pool(name="hbuf", bufs=1) as hpool,
            tc.tile_pool(name="psA", bufs=2, space="PSUM") as psA,
            tc.tile_pool(name="psT", bufs=2, space="PSUM") as psT,
            tc.tile_pool(name="psE", bufs=2, space="PSUM") as psE,
            tc.tile_pool(name="psF", bufs=2, space="PSUM") as psF,
        ):
            ones1 = cpool.tile([1, P], bf16)
            nc.vector.memset(ones1[:], 1.0)
            ones1f = cpool.tile([1, P], f32)
            nc.vector.memset(ones1f[:], 1.0)
            ident = cpool.tile([P, P], bf16)
            make_identity(nc, ident[:])
            a1t = cpool.tile([P, dd], bf16)
            nc.sync.dma_start(out=a1t[:], in_=a1r[:])
            a2t = cpool.tile([P, dd], bf16)
            nc.sync.dma_start(out=a2t[:], in_=a2r[:])
            pw = cpool.tile([P, 5], f32)
            nc.sync.dma_start(out=pw[:], in_=pwp[:])

            def load_w(src_t, kdim, nm):
                t0 = wpool.tile([P, dd], bf16, tag=nm + "0")
                t1 = wpool.tile([P, dd], bf16, tag=nm + "1")
                t2 = wpool.tile([1, dd], bf16, tag=nm + "2")
                nc.sync.dma_start(out=t0[:], in_=src_t[0:P, :])
                nc.sync.dma_start(out=t1[:], in_=src_t[P:2 * P, :])
                nc.sync.dma_start(out=t2[:], in_=src_t[kdim:kdim + 1, :])
                return t0, t1, t2

            w1s_t = load_w(w1s, emb, "w1s")
            w1d_t = load_w(w1d, emb, "w1d")
            w2s_t = load_w(w2s, dd, "w2s")
            w2d_t = load_w(w2d, dd, "w2d")

            def mm_rows(x0, x1, m0, wtile, psum):
                nc.tensor.matmul(out=psum[:], lhsT=x0[:, m0:m0 + P],
                                 rhs=wtile[0][:], start=True, stop=False)
                nc.tensor.matmul(out=psum[:], lhsT=x1[:, m0:m0 + P],
                                 rhs=wtile[1][:], start=False, stop=False)
                nc.tensor.matmul(out=psum[:], lhsT=ones1[:], rhs=wtile[2][:],
                                 start=False, stop=True)

            # ---------- phase A: fd1 (local) first, then fs1a, then fs1b ----
            for b in range(_ceil(npc_pad, BLK)):
                w = min(BLK, npc_pad - b * BLK)
                x0 = xpool.tile([P, BLK], bf16, tag="x0")
                x1 = xpool.tile([P, BLK], bf16, tag="x1")
                nc.sync.dma_start(out=x0[:, :w], in_=xTl[0:P, b * BLK:b * BLK + w])
                nc.sync.dma_start(out=x1[:, :w], in_=xTl[P:2 * P, b * BLK:b * BLK + w])
                for m in range(w // P):
                    row0 = b * BLK + m * P
                    ps = psA.tile([P, dd], f32, space="PSUM", tag="psa")
                    mm_rows(x0, x1, m * P, w1d_t, ps)
                    ot = mpool.tile([P, dd], bf16, tag="fsout")
                    nc.scalar.copy(out=ot[:], in_=ps[:])
                    nc.sync.dma_start(out=fd1[row0:row0 + P, :], in_=ot[:])

            for part in (0, 1):
                lo = 0 if part == 0 else rowsA
                hi = rowsA if part == 0 else nfull
                for b in range(lo // BLK, _ceil(hi, BLK)):
                    w = min(BLK, nfull - b * BLK)
                    x0 = xpool.tile([P, BLK], bf16, tag="x0")
                    x1 = xpool.tile([P, BLK], bf16, tag="x1")
                    nc.sync.dma_start(out=x0[:, :w],
                                      in_=xTa[0:P, b * BLK:b * BLK + w])
                    nc.sync.dma_start(out=x1[:, :w],
                                      in_=xTa[P:2 * P, b * BLK:b * BLK + w])
                    for m in range(w // P):
                        row0 = b * BLK + m * P
                        if (row0 < rowsA) != (part == 0):
                            continue
                        ps = psA.tile([P, dd], f32, space="PSUM", tag="psa")
                        mm_rows(x0, x1, m * P, w1s_t, ps)
                        ot = mpool.tile([P, dd], bf16, tag="fsout")
                        nc.scalar.copy(out=ot[:], in_=ps[:])
                        if row0 < rowsA:
                            nc.sync.dma_start(out=fs1a[row0:row0 + P, :],
                                              in_=ot[:])
                        else:
                            r = row0 - rowsA
                            nc.sync.dma_start(out=fs1b[r:r + P, :], in_=ot[:])

            # ---------- edge phase ----------
            h1 = hpool.tile([P, ntile, dd], bf16, tag="h")
            h2 = hpool.tile([P, ntile, dd], bf16, tag="h")

            def edge_phase(tabA, tabB, tabF, a_t, hdst, scale_posw, depth="all"):
                offA = offB = offC = 0
                for t in range(ntile):
                    ca, cb, ct = cA[t], cB[t], C[t]
                    St = s2pool.tile([P, Cmax * P], bf16, tag="S")
                    nc.sync.dma_start(
                        out=St[:, : ct * P],
                        in_=S_in[:, offC * P:(offC + ct) * P])
                    E = epool.tile([P, Cmax, dd], bf16, tag="E")
                    if ca:
                        ia = spool.tile([P, Cmax * 8], i16, tag="ia")
                        nc.sync.dma_start(
                            out=ia[:, : ca * 8],
                            in_=iA_in[:, offA * 8:(offA + ca) * 8])
                        nc.gpsimd.dma_gather(
                            out_ap=E[:, 0:ca, :], in_ap=tabA,
                            idxs_ap=ia[:, : ca * 8], num_idxs=ca * P,
                            num_idxs_reg=ca * P, elem_size=dd,
                            single_packet=False, queue_num=(2 * t) % 4)
                    if cb:
                        ib = spool.tile([P, Cmax * 8], i16, tag="ib")
                        nc.sync.dma_start(
                            out=ib[:, : cb * 8],
                            in_=iB_in[:, offB * 8:(offB + cb) * 8])
                        nc.gpsimd.dma_gather(
                            out_ap=E[:, ca:ct, :], in_ap=tabB,
                            idxs_ap=ib[:, : cb * 8], num_idxs=cb * P,
                            num_idxs_reg=cb * P, elem_size=dd,
                            single_packet=False, queue_num=(2 * t + 1) % 4)
                    # fd broadcast: fd rows for this tile's 128 dsts, expanded
                    # to edge slots via the transposed one-hot (PE matmul)
                    STt = s2pool.tile([P, Cmax * P], bf16, tag="ST")
                    nc.sync.dma_start(
                        out=STt[:, : ct * P],
                        in_=ST_in[:, offC * P:(offC + ct) * P])
                    fdt = spool.tile([P, dd], bf16, tag="fdt")
                    nc.sync.dma_start(out=fdt[:], in_=tabF[t * P:(t + 1) * P, :])
                    w_ = ct * dd
                    LZ = zpool.tile([P, Cmax * dd], bf16, tag="LZ")
                    for c in range(ct):
                        psf = psF.tile([P, dd], f32, space="PSUM", tag="psf")
                        nc.tensor.matmul(
                            out=psf[:], lhsT=STt[:, c * P:(c + 1) * P],
                            rhs=fdt[:], start=True, stop=False)
                        nc.tensor.matmul(
                            out=psf[:], lhsT=ident[:], rhs=E[:, c, :],
                            start=False, stop=True)
                        nc.scalar.activation(
                            LZ[:, c * dd:(c + 1) * dd], psf[:], AF.Prelu,
                            alpha=NEG_SLOPE)

                    if depth == "g":
                        nc.vector.memset(hdst[:, t, :], 0.0)
                        offA += ca; offB += cb; offC += ct
                        continue
                    T = zpool.tile([P, Cmax * dd], bf16, tag="T")
                    nc.vector.tensor_mul(
                        out=T[:, :w_].rearrange("p (c d) -> p c d", d=dd),
                        in0=LZ[:, :w_].rearrange("p (c d) -> p c d", d=dd),
                        in1=_bcast_mid(a_t[:], ct))
                    score = spool.tile([P, Cmax * H], f32, tag="sc")
                    nc.vector.reduce_sum(
                        out=score[:, : ct * H],
                        in_=T[:, :w_].rearrange("p (ch f) -> p ch f", f=F),
                        axis=mybir.AxisListType.X)
                    EX = spool.tile([P, Cmax * H], f32, tag="ex")
                    nc.scalar.activation(EX[:, : ct * H], score[:, : ct * H],
                                         AF.Exp)
                    if depth == "dve":
                        nc.vector.memset(hdst[:, t, :], 0.0)
                        offA += ca; offB += cb; offC += ct
                        continue

                    V = vpool.tile([P, Cmax, dd + H], bf16, tag="V")
                    exs = EX[:, : ct * H]
                    nc.vector.tensor_mul(
                        out=V[:, 0:ct, 0:dd].rearrange("p c (h f) -> p c h f",
                                                       f=F),
                        in0=E[:, 0:ct, :].rearrange("p c (h f) -> p c h f",
                                                    f=F),
                        in1=AP(exs.tensor, exs.offset,
                               [exs.ap[0], [H, ct], [1, H], [0, F]]))
                    nc.scalar.copy(
                        out=V[:, 0:ct, dd:dd + H],
                        in_=EX[:, : ct * H].rearrange("p (c h) -> p c h", h=H))

                    if depth == "v":
                        nc.vector.memset(hdst[:, t, :], 0.0)
                        offA += ca; offB += cb; offC += ct
                        continue
                    agg = psE.tile([P, dd + H], f32, space="PSUM", tag="agg")
                    for c in range(ct):
                        nc.tensor.matmul(
                            out=agg[:], lhsT=St[:, c * P:(c + 1) * P],
                            rhs=V[:, c, :], start=(c == 0), stop=(c == ct - 1))

                    den = spool.tile([P, H], f32, tag="den")
                    nc.vector.tensor_scalar_max(den[:], agg[:, dd:dd + H], 1e-9)
                    rec = spool.tile([P, H], f32, tag="rec")
                    nc.vector.reciprocal(rec[:], den[:])
                    if scale_posw:
                        nc.vector.tensor_scalar_mul(rec[:], rec[:],
                                                    pw[:, t % 5:t % 5 + 1])
                    for h in range(H):
                        nc.vector.tensor_scalar_mul(
                            hdst[:, t, h * F:(h + 1) * F],
                            agg[:, h * F:(h + 1) * F], rec[:, h:h + 1])
                    offA += ca
                    offB += cb
                    offC += ct

            if stop_after != "A":
                depth = {"L1g": "g", "L1dve": "dve", "L1v": "v"}.get(
                    stop_after, "all")
                edge_phase(fs1a[:], fs1b[:], fd1, a1t, h1, False, depth)

            # ---------- transpose h1 ----------
            hT0 = hpool.tile([P, npc_pad], bf16, tag="t0")
            hT1 = hpool.tile([P, npc_pad], bf16, tag="t1")
            for t in range(ntile if stop_after in ("TR", "FS2", "AG", "full") else 0):
                for half, ht in ((0, hT0), (1, hT1)):
                    pt = psT.tile([P, P], bf16, space="PSUM", tag="ptr")
                    nc.tensor.transpose(
                        out=pt[:], in_=h1[:, t, half * P:(half + 1) * P],
                        identity=ident[:])
                    nc.scalar.copy(out=ht[:, t * P:(t + 1) * P], in_=pt[:])

            # ---------- fs2 / fd2 local ----------
            for t in range(ntile if stop_after in ("FS2", "AG", "full") else 0):
                for wt, ob in ((w2s_t, "s"), (w2d_t, "d")):
                    ps = psA.tile([P, dd], f32, space="PSUM", tag="psa")
                    nc.tensor.matmul(out=ps[:], lhsT=hT0[:, t * P:(t + 1) * P],
                                     rhs=wt[0][:], start=True, stop=False)
                    nc.tensor.matmul(out=ps[:], lhsT=hT1[:, t * P:(t + 1) * P],
                                     rhs=wt[1][:], start=False, stop=False)
                    nc.tensor.matmul(out=ps[:], lhsT=ones1[:], rhs=wt[2][:],
                                     start=False, stop=True)
                    ot = mpool.tile([P, dd], bf16, tag="fsout")
                    nc.scalar.copy(out=ot[:], in_=ps[:])
                    if ob == "s":
                        hi = min((t + 1) * P, npc)
                        if hi > t * P:
                            nc.sync.dma_start(out=fs2l[t * P:hi, :],
                                              in_=ot[: hi - t * P, :])
                    else:
                        nc.sync.dma_start(out=fd2[t * P:(t + 1) * P, :], in_=ot[:])

            # ---------- AllGather fs2 ----------
            do_rest = stop_after in ("AG", "full")
            if do_rest:
                nc.gpsimd.collective_compute(
                "AllGather", ALU.bypass,
                    replica_groups=[list(range(NCORES))],
                    ins=[fs2l[:]], outs=[fs2f[0:n_nodes, :]])

            if do_rest and stop_after == "AG":
                do_rest = False
            if do_rest and use_B:
                edge_phase(fs2f[0:SPLIT, :], fs2f[SPLIT:nfull_ag_pad, :],
                           fd2, a2t, h2, True)
            elif do_rest:
                edge_phase(fs2f[0:nfull_ag_pad, :], fs2f[:], fd2, a2t, h2, True)

            # ---------- transpose h2 ----------
            gT0 = hpool.tile([P, npc_pad], bf16, tag="t0")
            gT1 = hpool.tile([P, npc_pad], bf16, tag="t1")
            for t in range(ntile if stop_after == "full" else 0):
                for half, ht in ((0, gT0), (1, gT1)):
                    pt = psT.tile([P, P], bf16, space="PSUM", tag="ptr")
                    nc.tensor.transpose(
                        out=pt[:], in_=h2[:, t, half * P:(half + 1) * P],
                        identity=ident[:])
                    nc.scalar.copy(out=ht[:, t * P:(t + 1) * P], in_=pt[:])

            # ---------- pooling + classifier ----------
            p0 = hpool.tile([P, npc // TOPK], f32, tag="p0")
            p1 = hpool.tile([P, npc // TOPK], f32, tag="p1")
            if stop_after == "full":
                nc.vector.reduce_sum(
                    out=p0[:],
                    in_=gT0[:, :npc].rearrange("p (g k) -> p g k", k=TOPK),
                    axis=mybir.AxisListType.X)
                nc.vector.reduce_sum(
                    out=p1[:],
                    in_=gT1[:, :npc].rearrange("p (g k) -> p g k", k=TOPK),
                    axis=mybir.AxisListType.X)
            else:
                nc.vector.memset(p0[:], 0.0)
                nc.vector.memset(p1[:], 0.0)

            wc0 = wpool.tile([P, ncls], f32, tag="wc0")
            wc1 = wpool.tile([P, ncls], f32, tag="wc1")
            wc2 = wpool.tile([1, ncls], f32, tag="wc2")
            nc.sync.dma_start(out=wc0[:], in_=wca[0:P, :])
            nc.sync.dma_start(out=wc1[:], in_=wca[P:2 * P, :])
            nc.sync.dma_start(out=wc2[:], in_=wca[dd:dd + 1, :])

            for g0 in range(0, ngrp, P):
                gw = min(P, ngrp - g0)
                pc_ = psA.tile([P, ncls], f32, space="PSUM", tag="psa")
                nc.tensor.matmul(out=pc_[:gw, :], lhsT=p0[:, g0:g0 + gw],
                                 rhs=wc0[:], start=True, stop=False)
                nc.tensor.matmul(out=pc_[:gw, :], lhsT=p1[:, g0:g0 + gw],
                                 rhs=wc1[:], start=False, stop=False)
                nc.tensor.matmul(out=pc_[:gw, :], lhsT=ones1f[:, :gw],
                                 rhs=wc2[:], start=False, stop=True)
                oc = mpool.tile([P, ncls], f32, tag="ocls")
                nc.scalar.copy(out=oc[:gw, :], in_=pc_[:gw, :])
                nc.sync.dma_start(out=out[g0:g0 + gw, :], in_=oc[:gw, :])

    nc.compile()
    return nc


def _build_inputs(inputs):
    x = np.asarray(inputs["x"], dtype=np.float32)
    src = np.asarray(inputs["src"]).astype(np.int64)
    dst = np.asarray(inputs["dst"]).astype(np.int64)
    n_nodes, emb = x.shape
    dd = np.asarray(inputs["w1_src"]).shape[1]
    H = np.asarray(inputs["a1"]).shape[0]
    ncls = np.asarray(inputs["wc"]).shape[1]

    per_core, cA, cB, C, ntile, npc = _prep_graph(src, dst, n_nodes)
    npc_pad = ntile * P
    nfull_pad = _ceil(n_nodes, P) * P

    meta = dict(n_nodes=n_nodes, npc=npc, ntile=ntile, cA=cA, cB=cB, C=C,
                emb=emb, d=dd, H=H, ncls=ncls, nfull_pad=nfull_pad)

    xT = np.zeros((emb + 1, nfull_pad), dtype=_bf)
    xT[:emb, :n_nodes] = x.T.astype(_bf)
    xT[emb, :] = _bf(1.0)
    w1s = _aug(inputs["w1_src"], inputs["b1_src"]).astype(_bf)
    w1d = _aug(inputs["w1_dst"], inputs["b1_dst"]).astype(_bf)
    w2s = _aug(inputs["w2_src"], inputs["b2_src"]).astype(_bf)
    w2d = _aug(inputs["w2_dst"], inputs["b2_dst"]).astype(_bf)
    a1rr = np.tile(np.asarray(inputs["a1"]).reshape(1, -1), (P, 1)).astype(_bf)
    a2rr = np.tile(np.asarray(inputs["a2"]).reshape(1, -1), (P, 1)).astype(_bf)
    wca = _aug(inputs["wc"], inputs["bc"]).astype(np.float32)
    pos_w = np.asarray(inputs["pos_w"], dtype=np.float32)
    pwp = np.zeros((P, 5), dtype=np.float32)
    for j in range(5):
        for p in range(P):
            pwp[p, j] = pos_w[(P * j + p) % TOPK]

    in_maps = []
    for c in range(NCORES):
        d = per_core[c]
        xl = np.zeros((emb + 1, npc_pad), dtype=_bf)
        xl[:emb, :npc] = x[c * npc:(c + 1) * npc].T.astype(_bf)
        xl[emb, :] = _bf(1.0)
        in_maps.append(dict(
            xTa=xT, xTl=xl, w1s=w1s, w1d=w1d, w2s=w2s, w2d=w2d,
            a1r=a1rr, a2r=a2rr, wca=wca, pwp=pwp,
            iA=np.ascontiguousarray(d["iA"]), iB=np.ascontiguousarray(d["iB"]),
            ST=np.ascontiguousarray(d["ST"]), S=np.ascontiguousarray(d["S"]),
        ))
    return meta, in_maps


def run(inputs, trace=False, stop_after="full", cores=None):
    meta, in_maps = _build_inputs(inputs)
    nc = build_nc(meta, stop_after=stop_after)
    ids = list(range(NCORES)) if cores is None else list(range(cores))
    res = run_bass_kernel_spmd(nc, [in_maps[c] for c in ids], core_ids=ids,
                               trace=trace)
    outs = [res.results[i]["out"] for i in range(len(ids))]
    return np.concatenate(outs, axis=0), res


def kernel(**inputs):
    out, _ = run(inputs, trace=False)
    return out



# revision 9
# speedup vs baseline: 1.2870x; 1.1794x over previous
"""Trainium2 Bass kernel for a 2-layer GATv2 + top-k pooling + classifier.

Distribution (8 NeuronCores): nodes (and their incoming edges) are
partitioned across cores; per-layer source tables are built locally and
exchanged with one AllGather per layer. GAT weights / classifier are
replicated.

Key structure:
 - The attention vector `a` is folded into the weight matrices host-side
   (columns scaled by |a| and permuted so positive-sign features come
   first per head); scores are then sign-block reduce_sums of
   leaky_relu(psi~), with exact compensation in the downstream weights.
 - All biases are folded into the dst-feature bias / downstream weights
   (valid because softmax weights sum to 1 and no dst node is isolated;
   a fallback flag handles the isolated-node case).
 - Edge phase per 128-dst tile: DMA-gather src rows, one-hot matmuls for
   dst broadcast + scatter aggregation, batched Prelu from PSUM.

Self-contained: only needs concourse (Bass), numpy, ml_dtypes.
"""

import numpy as np
import ml_dtypes

import concourse.bass as bass
import concourse.bacc as bacc
import concourse.mybir as mybir
import concourse.tile as tile
from concourse.bass import AP
from concourse.bass_utils import run_bass_kernel_spmd
from concourse.masks import make_identity

P = 128
NCORES = 8
SPLIT = 32768          # int16 gather index limit per table
NEG_SLOPE = 0.2
TOPK = 10
BLK = 1024

bf16 = mybir.dt.bfloat16
f32 = mybir.dt.float32
i16 = mybir.dt.int16

_bf = ml_dtypes.bfloat16


def _wrap_idx(idx):
    """Pack int16 indices into dma_gather's [128, n//16] SBUF layout."""
    n = idx.shape[0]
    assert n % 16 == 0
    t = idx.astype(np.int16).reshape(n // 16, 16).T
    return np.tile(t, (8, 1))


def _pad(a, n, val=0):
    out = np.full(n, val, dtype=np.int64)
    out[: len(a)] = a
    return out


def _ceil(a, b):
    return -(-a // b)


def _prep_graph(src, dst, n_nodes):
    """Partition edges by dst core, group by 128-dst tile, split sources
    at SPLIT. Returns per-core packed gather-index + one-hot inputs plus
    shared per-tile chunk counts."""
    npc = n_nodes // NCORES
    ntile = _ceil(npc, P)
    core_of = dst // npc

    pc = []
    for c in range(NCORES):
        m = core_of == c
        es = src[m].astype(np.int64)
        ed = dst[m].astype(np.int64) - c * npc
        o = np.argsort(ed, kind="stable")
        es, ed = es[o], ed[o]
        bounds = np.searchsorted(ed, np.arange(0, ntile * P + 1, P))
        tiles = []
        for t in range(ntile):
            sl = slice(bounds[t], bounds[t + 1])
            ts_, td_ = es[sl], ed[sl]
            am = ts_ < SPLIT
            tiles.append(dict(
                a_src=ts_[am], a_col=td_[am] - t * P,
                b_src=ts_[~am] - SPLIT, b_col=td_[~am] - t * P,
            ))
        pc.append(tiles)

    cA = [max(_ceil(len(pc[c][t]["a_src"]), P) for c in range(NCORES))
          for t in range(ntile)]
    cB = [max(_ceil(len(pc[c][t]["b_src"]), P) for c in range(NCORES))
          for t in range(ntile)]
    C = [cA[t] + cB[t] for t in range(ntile)]

    per_core = []
    for c in range(NCORES):
        iA, iB, SST = [], [], []
        for t in range(ntile):
            d = pc[c][t]
            nA, nB = len(d["a_src"]), len(d["b_src"])
            if cA[t]:
                iA.append(_wrap_idx(_pad(d["a_src"], cA[t] * P)))
            if cB[t]:
                iB.append(_wrap_idx(_pad(d["b_src"], cB[t] * P)))
            S3 = np.zeros((C[t] * P, P), dtype=np.float32)
            S3[np.arange(nA), d["a_col"]] = 1.0
            S3[cA[t] * P + np.arange(nB), d["b_col"]] = 1.0
            St = S3.reshape(C[t], P, P).transpose(1, 0, 2).reshape(P, -1)
            STt = S3.reshape(C[t], P, P).transpose(2, 0, 1).reshape(P, -1)
            SST.append(np.concatenate([St, STt], axis=1))
        per_core.append(dict(
            iA=np.concatenate(iA, axis=1) if iA else np.zeros((P, 8), np.int16),
            iB=np.concatenate(iB, axis=1) if iB else np.zeros((P, 8), np.int16),
            SST=np.concatenate(SST, axis=1).astype(_bf),
        ))
    return per_core, cA, cB, C, ntile, npc


def build_nc(meta):
    n_nodes = meta["n_nodes"]
    npc = meta["npc"]
    ntile = meta["ntile"]
    cA, cB, C = meta["cA"], meta["cB"], meta["C"]
    Cmax = max(C)
    sumA, sumB, sumC = sum(cA), sum(cB), sum(C)
    emb = meta["emb"]
    dd = meta["d"]
    H = meta["H"]
    F = dd // H
    ncls = meta["ncls"]
    k1, k2 = meta["k1"], meta["k2"]
    npc_pad = ntile * P
    ngrp = npc // TOPK
    nfull_ag_pad = _ceil(n_nodes, P) * P

    nc = bacc.Bacc(num_swdge_queues=4)

    xTl = nc.declare_dram_parameter("xTl", [emb, npc_pad], bf16, isOutput=False)
    w1s = nc.declare_dram_parameter("w1s", [emb, dd], bf16, isOutput=False)
    w1d = nc.declare_dram_parameter("w1d", [emb, dd], bf16, isOutput=False)
    w2s = nc.declare_dram_parameter("w2s", [dd, dd], bf16, isOutput=False)
    w2d = nc.declare_dram_parameter("w2d", [dd, dd], bf16, isOutput=False)
    brep = nc.declare_dram_parameter("brep", [P, 2 * dd], bf16, isOutput=False)
    wca = nc.declare_dram_parameter("wca", [dd + 1, ncls], f32, isOutput=False)
    pwp = nc.declare_dram_parameter("pwp", [P, 5], f32, isOutput=False)
    iA_in = nc.declare_dram_parameter("iA", [P, max(sumA, 1) * 8], i16, isOutput=False)
    iB_in = nc.declare_dram_parameter("iB", [P, max(sumB, 1) * 8], i16, isOutput=False)
    SST_in = nc.declare_dram_parameter("SST", [P, 2 * sumC * P], bf16, isOutput=False)
    out = nc.declare_dram_parameter("out", [ngrp, ncls], f32, isOutput=True)

    fs1l = nc.dram_tensor("fs1l", [npc, dd], bf16)
    fs2l = nc.dram_tensor("fs2l", [npc, dd], bf16)
    fs1f = nc.dram_tensor("fs1f", [nfull_ag_pad, dd], bf16, addr_space="Shared")
    fs2f = nc.dram_tensor("fs2f", [nfull_ag_pad, dd], bf16, addr_space="Shared")

    AF = mybir.ActivationFunctionType
    ALU = mybir.AluOpType
    X = mybir.AxisListType.X

    with tile.TileContext(nc) as tc:
        with (
            tc.tile_pool(name="const", bufs=1) as cpool,
            tc.tile_pool(name="wpool", bufs=1) as wpool,
            tc.tile_pool(name="xload", bufs=2) as xpool,
            tc.tile_pool(name="mmout", bufs=3) as mpool,
            tc.tile_pool(name="edgeg", bufs=3) as epool,
            tc.tile_pool(name="sstp", bufs=2) as sstpool,
            tc.tile_pool(name="zp", bufs=2) as zpool,
            tc.tile_pool(name="vp", bufs=2) as vpool,
            tc.tile_pool(name="small", bufs=3) as spool,
            tc.tile_pool(name="hbuf", bufs=1) as hpool,
            tc.tile_pool(name="psA", bufs=2, space="PSUM") as psA,
            tc.tile_pool(name="psF", bufs=2, space="PSUM") as psF,
            tc.tile_pool(name="psE", bufs=2, space="PSUM") as psE,
            tc.tile_pool(name="psT", bufs=2, space="PSUM") as psT,
        ):
            ident = cpool.tile([P, P], bf16)
            make_identity(nc, ident[:])
            ones1f = cpool.tile([1, P], f32)
            nc.vector.memset(ones1f[:], 1.0)
            pw = cpool.tile([P, 5], f32)
            nc.sync.dma_start(out=pw[:], in_=pwp[:])
            bt = cpool.tile([P, 2, dd], bf16)
            nc.sync.dma_start(out=bt[:], in_=brep[:].rearrange(
                "p (b d) -> p b d", d=dd))
            iasb = cpool.tile([P, max(sumA, 1) * 8], i16)
            nc.sync.dma_start(out=iasb[:], in_=iA_in[:])
            ibsb = cpool.tile([P, max(sumB, 1) * 8], i16)
            nc.sync.dma_start(out=ibsb[:], in_=iB_in[:])

            def load_w(src_t, nm):
                t0 = wpool.tile([P, dd], bf16, tag=nm + "0")
                t1 = wpool.tile([P, dd], bf16, tag=nm + "1")
                nc.sync.dma_start(out=t0[:], in_=src_t[0:P, :])
                nc.sync.dma_start(out=t1[:], in_=src_t[P:2 * P, :])
                return t0, t1

            w1s_t = load_w(w1s, "w1s")
            w1d_t = load_w(w1d, "w1d")
            w2s_t = load_w(w2s, "w2s")
            w2d_t = load_w(w2d, "w2d")

            wc0 = wpool.tile([P, ncls], f32, tag="wc0")
            wc1 = wpool.tile([P, ncls], f32, tag="wc1")
            wc2 = wpool.tile([1, ncls], f32, tag="wc2")
            nc.sync.dma_start(out=wc0[:], in_=wca[0:P, :])
            nc.sync.dma_start(out=wc1[:], in_=wca[P:2 * P, :])
            nc.sync.dma_start(out=wc2[:], in_=wca[dd:dd + 1, :])

            # ---------------- dense helpers ----------------
            def dense_x_pass(wt, sink):
                for b in range(_ceil(npc_pad, BLK)):
                    w = min(BLK, npc_pad - b * BLK)
                    x0 = xpool.tile([P, BLK], bf16, tag="x0")
                    x1 = xpool.tile([P, BLK], bf16, tag="x1")
                    nc.sync.dma_start(out=x0[:, :w],
                                      in_=xTl[0:P, b * BLK:b * BLK + w])
                    nc.sync.dma_start(out=x1[:, :w],
                                      in_=xTl[P:2 * P, b * BLK:b * BLK + w])
                    for m in range(w // P):
                        row0 = b * BLK + m * P
                        ps = psA.tile([P, dd], f32, space="PSUM", tag="psa")
                        nc.tensor.matmul(out=ps[:], lhsT=x0[:, m * P:(m + 1) * P],
                                         rhs=wt[0][:], start=True, stop=False)
                        nc.tensor.matmul(out=ps[:], lhsT=x1[:, m * P:(m + 1) * P],
                                         rhs=wt[1][:], start=False, stop=True)
                        sink(row0, ps)

            def dense_h_pass(hT, wt, sink):
                for t in range(ntile):
                    ps = psA.tile([P, dd], f32, space="PSUM", tag="psa")
                    nc.tensor.matmul(out=ps[:], lhsT=hT[:, 0, t * P:(t + 1) * P],
                                     rhs=wt[0][:], start=True, stop=False)
                    nc.tensor.matmul(out=ps[:], lhsT=hT[:, 1, t * P:(t + 1) * P],
                                     rhs=wt[1][:], start=False, stop=True)
                    sink(t * P, ps)

            def fs_sink(dramt):
                def s(row0, ps):
                    ot = mpool.tile([P, dd], bf16, tag="ot")
                    nc.scalar.copy(out=ot[:], in_=ps[:])
                    hi = min(row0 + P, npc)
                    if hi > row0:
                        nc.sync.dma_start(out=dramt[row0:hi, :],
                                          in_=ot[: hi - row0, :])
                return s

            def fd_sink(fdt, bidx):
                def s(row0, ps):
                    t = row0 // P
                    nc.vector.tensor_add(out=fdt[:, t, :], in0=ps[:],
                                         in1=bt[:, bidx, :])
                return s

            # ---------------- edge phase ----------------
            offs = []
            offA = offB = offC = 0
            for t in range(ntile):
                offs.append((offA, offB, offC))
                offA += cA[t]
                offB += cB[t]
                offC += C[t]

            def edge_phase(tabA, tabB, fdt, kvec, scale_posw, hdst):
                ctx = {}

                def stageA(t):
                    oA, oB, oC = offs[t]
                    ca, cb, ct = cA[t], cB[t], C[t]
                    sst = sstpool.tile([P, 2 * Cmax * P], bf16, tag="SST")
                    nc.sync.dma_start(
                        out=sst[:, : 2 * ct * P],
                        in_=SST_in[:, 2 * oC * P:2 * (oC + ct) * P])
                    E = epool.tile([P, Cmax, dd], bf16, tag="E")
                    if ca:
                        nc.gpsimd.dma_gather(
                            out_ap=E[:, 0:ca, :], in_ap=tabA,
                            idxs_ap=iasb[:, oA * 8:(oA + ca) * 8],
                            num_idxs=ca * P, num_idxs_reg=ca * P,
                            elem_size=dd, single_packet=False,
                            queue_num=(2 * t) % 4)
                    if cb:
                        nc.gpsimd.dma_gather(
                            out_ap=E[:, ca:ct, :], in_ap=tabB,
                            idxs_ap=ibsb[:, oB * 8:(oB + cb) * 8],
                            num_idxs=cb * P, num_idxs_reg=cb * P,
                            elem_size=dd, single_packet=False,
                            queue_num=(2 * t + 1) % 4)
                    LZ = zpool.tile([P, Cmax * dd], bf16, tag="LZ")
                    for j in range(0, ct, 2):
                        n = min(2, ct - j)
                        psf = psF.tile([P, 2 * dd], f32, space="PSUM", tag="psf")
                        for i in range(n):
                            c = j + i
                            nc.tensor.matmul(
                                out=psf[:, i * dd:(i + 1) * dd],
                                lhsT=sst[:, (ct + c) * P:(ct + c + 1) * P],
                                rhs=fdt[:, t, :], start=True, stop=False)
                            nc.tensor.matmul(
                                out=psf[:, i * dd:(i + 1) * dd],
                                lhsT=ident[:], rhs=E[:, c, :],
                                start=False, stop=True)
                        nc.scalar.activation(LZ[:, j * dd:(j + n) * dd],
                                             psf[:, : n * dd], AF.Prelu,
                                             alpha=NEG_SLOPE)
                    ctx[t] = (sst, E, LZ)

                def stageB(t):
                    sst, E, LZ = ctx.pop(t)
                    ca, cb, ct = cA[t], cB[t], C[t]
                    r = spool.tile([P, Cmax * H * 2], f32, tag="r")
                    LZ3 = LZ[:, : ct * dd].rearrange("p (c d) -> p c d", d=dd)
                    r4 = r[:, : ct * H * 2].rearrange("p (c h s) -> p c h s",
                                                      h=H, s=2)
                    for h in range(H):
                        k = int(kvec[h])
                        for s_, lo, hi in ((0, h * F, h * F + k),
                                           (1, h * F + k, (h + 1) * F)):
                            if hi > lo:
                                nc.vector.reduce_sum(out=r4[:, :, h, s_],
                                                     in_=LZ3[:, :, lo:hi],
                                                     axis=X)
                            else:
                                nc.vector.memset(r4[:, :, h, s_], 0.0)
                    sc = spool.tile([P, Cmax * H], f32, tag="sc")
                    nc.vector.tensor_sub(
                        out=sc[:, : ct * H].rearrange("p (c h) -> p c h", h=H),
                        in0=r4[:, :, :, 0], in1=r4[:, :, :, 1])
                    ex = spool.tile([P, Cmax * H], bf16, tag="ex")
                    nc.scalar.activation(ex[:, : ct * H], sc[:, : ct * H],
                                         AF.Exp)
                    V = vpool.tile([P, Cmax, dd + H], bf16, tag="V")
                    exs = ex[:, : ct * H]
                    nc.vector.tensor_mul(
                        out=V[:, 0:ct, 0:dd].rearrange("p c (h f) -> p c h f",
                                                       f=F),
                        in0=E[:, 0:ct, :].rearrange("p c (h f) -> p c h f",
                                                    f=F),
                        in1=AP(exs.tensor, exs.offset,
                               [exs.ap[0], [H, ct], [1, H], [0, F]]))
                    nc.scalar.copy(
                        out=V[:, 0:ct, dd:dd + H],
                        in_=exs.rearrange("p (c h) -> p c h", h=H))
                    agg = psE.tile([P, dd + H], f32, space="PSUM", tag="agg")
                    for c in range(ct):
                        nc.tensor.matmul(out=agg[:],
                                         lhsT=sst[:, c * P:(c + 1) * P],
                                         rhs=V[:, c, :], start=(c == 0),
                                         stop=(c == ct - 1))
                    den = spool.tile([P, H], f32, tag="den")
                    nc.vector.tensor_scalar_max(den[:], agg[:, dd:dd + H], 1e-9)
                    rec = spool.tile([P, H], f32, tag="rec")
                    nc.vector.reciprocal(rec[:], den[:])
                    if scale_posw:
                        nc.vector.tensor_scalar_mul(rec[:], rec[:],
                                                    pw[:, t % 5:t % 5 + 1])
                    recs = rec[:]
                    nc.vector.tensor_mul(
                        out=hdst[:, t, :].rearrange("p (h f) -> p h f", f=F),
                        in0=agg[:, 0:dd].rearrange("p (h f) -> p h f", f=F),
                        in1=AP(recs.tensor, recs.offset,
                               [recs.ap[0], [1, H], [0, F]]))

                for t in range(ntile):
                    stageA(t)
                    if t > 0:
                        stageB(t - 1)
                stageB(ntile - 1)

            def transpose_h(h, hT):
                for t in range(ntile):
                    pt = psT.tile([P, 2, P], bf16, space="PSUM", tag="pt")
                    nc.tensor.transpose(out=pt[:, 0, :], in_=h[:, t, 0:P],
                                        identity=ident[:])
                    nc.tensor.transpose(out=pt[:, 1, :], in_=h[:, t, P:2 * P],
                                        identity=ident[:])
                    nc.vector.tensor_copy(out=hT[:, :, t * P:(t + 1) * P],
                                          in_=pt[:])

            # ---------------- layer 1 ----------------
            dense_x_pass(w1s_t, fs_sink(fs1l))
            nc.gpsimd.collective_compute(
                "AllGather", ALU.bypass,
                replica_groups=[list(range(NCORES))],
                ins=[fs1l[:]], outs=[fs1f[0:n_nodes, :]])
            fd1 = hpool.tile([P, ntile, dd], bf16, tag="fd")
            dense_x_pass(w1d_t, fd_sink(fd1, 0))

            rowsA = min(nfull_ag_pad, SPLIT)
            h1 = hpool.tile([P, ntile, dd], bf16, tag="h")
            edge_phase(fs1f[0:rowsA, :],
                       fs1f[rowsA:nfull_ag_pad, :] if nfull_ag_pad > rowsA
                       else fs1f[:],
                       fd1, k1, False, h1)

            hT = hpool.tile([P, 2, npc_pad], bf16, tag="hT")
            transpose_h(h1, hT)

            # ---------------- layer 2 ----------------
            dense_h_pass(hT, w2s_t, fs_sink(fs2l))
            nc.gpsimd.collective_compute(
                "AllGather", ALU.bypass,
                replica_groups=[list(range(NCORES))],
                ins=[fs2l[:]], outs=[fs2f[0:n_nodes, :]])
            fd2 = hpool.tile([P, ntile, dd], bf16, tag="fd")
            dense_h_pass(hT, w2d_t, fd_sink(fd2, 1))

            h2 = hpool.tile([P, ntile, dd], bf16, tag="h")
            edge_phase(fs2f[0:rowsA, :],
                       fs2f[rowsA:nfull_ag_pad, :] if nfull_ag_pad > rowsA
                       else fs2f[:],
                       fd2, k2, True, h2)

            gT = hpool.tile([P, 2, npc_pad], bf16, tag="hT")
            transpose_h(h2, gT)

            # ---------------- pooling + classifier ----------------
            pp = hpool.tile([P, 2, ngrp], f32, tag="pp")
            for half in (0, 1):
                nc.vector.reduce_sum(
                    out=pp[:, half, :],
                    in_=gT[:, half, 0:npc].rearrange("p (g k) -> p g k",
                                                     k=TOPK),
                    axis=X)

            for g0 in range(0, ngrp, P):
                gw = min(P, ngrp - g0)
                pc_ = psA.tile([P, ncls], f32, space="PSUM", tag="psa")
                nc.tensor.matmul(out=pc_[:gw, :], lhsT=pp[:, 0, g0:g0 + gw],
                                 rhs=wc0[:], start=True, stop=False)
                nc.tensor.matmul(out=pc_[:gw, :], lhsT=pp[:, 1, g0:g0 + gw],
                                 rhs=wc1[:], start=False, stop=False)
                nc.tensor.matmul(out=pc_[:gw, :], lhsT=ones1f[:, :gw],
                                 rhs=wc2[:], start=False, stop=True)
                oc = mpool.tile([P, ncls], f32, tag="ocls")
                nc.scalar.copy(out=oc[:gw, :], in_=pc_[:gw, :])
                nc.sync.dma_start(out=out[g0:g0 + gw, :], in_=oc[:gw, :])

    nc.compile()
    return nc


def _fold_weights(inputs, H):
    """Fold |a| scaling + sign permutation into weights; fold all biases
    into the dst-path bias / downstream weights (see module docstring)."""
    a1 = np.asarray(inputs["a1"], np.float64).reshape(-1)
    a2 = np.asarray(inputs["a2"], np.float64).reshape(-1)
    dd = a1.shape[0]
    F = dd // H

    def perm_of(a):
        p, k = [], []
        for h in range(H):
            seg = a[h * F:(h + 1) * F]
            pos = np.where(seg >= 0)[0] + h * F
            neg = np.where(seg < 0)[0] + h * F
            p.extend(pos.tolist())
            p.extend(neg.tolist())
            k.append(len(pos))
        return np.array(p), k

    p1, k1 = perm_of(a1)
    p2, k2 = perm_of(a2)
    s1 = np.maximum(np.abs(a1[p1]), 1e-6)
    s2 = np.maximum(np.abs(a2[p2]), 1e-6)

    W1s = np.asarray(inputs["w1_src"], np.float64)
    b1s = np.asarray(inputs["b1_src"], np.float64)
    W1d = np.asarray(inputs["w1_dst"], np.float64)
    b1d = np.asarray(inputs["b1_dst"], np.float64)
    W2s = np.asarray(inputs["w2_src"], np.float64)
    b2s = np.asarray(inputs["b2_src"], np.float64)
    W2d = np.asarray(inputs["w2_dst"], np.float64)
    b2d = np.asarray(inputs["b2_dst"], np.float64)
    wc = np.asarray(inputs["wc"], np.float64)
    bc = np.asarray(inputs["bc"], np.float64)

    w1s_dev = W1s[:, p1] * s1[None, :]
    w1d_dev = W1d[:, p1] * s1[None, :]
    B1 = s1 * (b1s + b1d)[p1]

    w2s_dev = (W2s[p1][:, p2] * s2[None, :]) / s1[:, None]
    w2d_dev = (W2d[p1][:, p2] * s2[None, :]) / s1[:, None]
    c2s = b1s @ W2s + b2s
    c2d = b1s @ W2d + b2d
    B2 = s2 * (c2s + c2d)[p2]

    wc_dev = wc[p2, :] / s2[:, None]
    bc_dev = bc + c2s @ wc

    return dict(w1s=w1s_dev, w1d=w1d_dev, w2s=w2s_dev, w2d=w2d_dev,
                B1=B1, B2=B2, wc=wc_dev, bc=bc_dev, k1=k1, k2=k2)


def _build_inputs(inputs):
    x = np.asarray(inputs["x"], dtype=np.float32)
    src = np.asarray(inputs["src"]).astype(np.int64)
    dst = np.asarray(inputs["dst"]).astype(np.int64)
    n_nodes, emb = x.shape
    dd = np.asarray(inputs["w1_src"]).shape[1]
    H = np.asarray(inputs["a1"]).shape[0]
    ncls = np.asarray(inputs["wc"]).shape[1]

    per_core, cA, cB, C, ntile, npc = _prep_graph(src, dst, n_nodes)
    npc_pad = ntile * P

    fw = _fold_weights(inputs, H)

    meta = dict(n_nodes=n_nodes, npc=npc, ntile=ntile, cA=cA, cB=cB, C=C,
                emb=emb, d=dd, H=H, ncls=ncls, k1=fw["k1"], k2=fw["k2"])

    w1s = fw["w1s"].astype(_bf)
    w1d = fw["w1d"].astype(_bf)
    w2s = fw["w2s"].astype(_bf)
    w2d = fw["w2d"].astype(_bf)
    brep = np.tile(np.concatenate([fw["B1"], fw["B2"]]).reshape(1, -1),
                   (P, 1)).astype(_bf)
    wca = np.vstack([fw["wc"], fw["bc"][None, :]]).astype(np.float32)
    pos_w = np.asarray(inputs["pos_w"], dtype=np.float32)
    pwp = np.zeros((P, 5), dtype=np.float32)
    for j in range(5):
        for p in range(P):
            pwp[p, j] = pos_w[(P * j + p) % TOPK]

    in_maps = []
    for c in range(NCORES):
        d = per_core[c]
        xl = np.zeros((emb, npc_pad), dtype=_bf)
        xl[:, :npc] = x[c * npc:(c + 1) * npc].T.astype(_bf)
        in_maps.append(dict(
            xTl=xl, w1s=w1s, w1d=w1d, w2s=w2s, w2d=w2d,
            brep=brep, wca=wca, pwp=pwp,
            iA=np.ascontiguousarray(d["iA"]), iB=np.ascontiguousarray(d["iB"]),
            SST=np.ascontiguousarray(d["SST"]),
        ))
    return meta, in_maps


def run(inputs, trace=False, cores=None):
    meta, in_maps = _build_inputs(inputs)
    nc = build_nc(meta)
    ids = list(range(NCORES)) if cores is None else list(range(cores))
    res = run_bass_kernel_spmd(nc, [in_maps[c] for c in ids], core_ids=ids,
                               trace=trace)
    outs = [res.results[i]["out"] for i in range(len(ids))]
    return np.concatenate(outs, axis=0), res


def kernel(**inputs):
    out, _ = run(inputs, trace=False)
    return out
